# revision 17
# baseline (speedup 1.0000x reference)
"""GAT layer on 8 Trainium2 NeuronCores (Bass/Tile, SPMD).

Sharding: nodes partitioned across the 8 cores; each edge is assigned to
the core owning its dst node, so edge-softmax and the scatter-sum are
core-local.  Weights are replicated and pre-composed on the host
(q = x @ (Wv Wq) + (bv Wq + bq), etc).  Each core computes the full node
table [h | q | k] (replicated compute instead of a halo all-gather), in a
per-core permutation that places its own dst nodes in segment-window
order.  Edges are processed in segments of <=128 dst nodes; h/q rows are
fetched with dma_gather (negative-index-masked two-pass for the int16
range) spread over the 4 SWDGE queues (Q7 core pairs), k[dst] is expanded
on the TensorEngine via the transposed one-hot, and the softmax-weighted
aggregation is a one-hot matmul into PSUM.

Perf structure: all per-segment metadata (gather indices, dst-relative
one-hot keys, k windows, outputs) lives in batched SBUF tiles loaded or
flushed in single DMAs, keeping the Sync HWDGE queue short.  The gather
train on the Pool engine is the critical path; A-half gathers start as
soon as the A-half of the table is written.
"""
import sys

for _p in ("/opt/trn_rl_repo",):
    if _p not in sys.path:
        sys.path.insert(0, _p)

import numpy as np
import ml_dtypes

import concourse.bass as bass
from concourse import bacc, tile, library_config
from concourse.tile import add_dep_helper
import concourse.mybir as mybir
from concourse.bass_utils import run_bass_kernel_spmd

F32 = mybir.dt.float32
BF16 = mybir.dt.bfloat16
I32 = mybir.dt.int32
I16 = mybir.dt.int16
BF = ml_dtypes.bfloat16

# problem constants
N = 50000
E = 800000
IN_F = 128
H = 8
F = 16
HF = H * F  # 128
C = 8            # cores
CAP = 12 * 128   # max edges per segment
KH = 7           # chunk slots per table half
KT = 2 * KH      # chunk slots per segment
ROW = 256        # table row units (bf16): [h 128 | q 8 | k 8 | pad]
XB = 2048        # phase-A x tile width
WB = 1024        # tab rows per write DMA
NL = N // C      # nodes per core


def _edges_of_core(src, dst, c):
    lo = c * NL
    m = (dst >= lo) & (dst < lo + NL)
    es = src[m].astype(np.int64)
    ed = (dst[m] - lo).astype(np.int64)
    order = np.lexsort((es, ed))
    es, ed = es[order], ed[order]
    counts = np.bincount(ed, minlength=NL)
    return es, ed, counts


def _segment(es, ed, counts, isA):
    """Greedy segmentation: window <=128 nodes, per-half <= KH*128 edges."""
    cum = np.concatenate([[0], np.cumsum(counts)])
    cumA = np.concatenate([[0], np.cumsum(isA.astype(np.int64))])[cum]
    segs = []
    n0 = 0
    HC = KH * 128
    while n0 < NL:
        e0 = cum[n0]
        a0 = cumA[n0]
        n_hi = min(n0 + 128, NL)
        # largest n1 with both halves within capacity
        n1 = n0
        for n in range(n0 + 1, n_hi + 1):
            na = cumA[n] - a0
            nb = (cum[n] - e0) - na
            if na > HC or nb > HC:
                break
            n1 = n
        if n1 == n0:
            raise ValueError("node exceeds half capacity")
        segs.append((int(e0), int(cum[n1]), n0, n1))
        n0 = n1
    return segs


def _core_arrays(es, ed, segs, nseg, c, split, npad):
    """Build per-core device arrays + the node permutation.

    Table rows (per core): [0, nseg*128) = segment-window slots of local
    nodes; [nseg*128, ...) = all other nodes; padded to npad.
    perm[row] = global node id occupying that row (-1 for pad slots).
    """
    lo = c * NL
    nloc = nseg * 128
    perm = np.full(npad, -1, np.int64)
    row_of_node = np.full(N, -1, np.int64)
    for si, (e0, e1, n0, n1) in enumerate(segs):
        nn = n1 - n0
        rows = si * 128 + np.arange(nn)
        perm[rows] = lo + n0 + np.arange(nn)
        row_of_node[lo + n0 + np.arange(nn)] = rows
    foreign = np.concatenate([np.arange(0, lo), np.arange(lo + NL, N)])
    perm[nloc:nloc + len(foreign)] = foreign
    row_of_node[foreign] = nloc + np.arange(len(foreign))

    # pad slots hold dummy VALID index 0 (row 0 fetched, killed by
    # dstrel=-1 downstream): every gather processes exactly KH*128 idxs,
    # keeping the NX decode's ring accounting and the Q7 ucode in lockstep
    # with the constant num_idxs register.
    idxw = np.zeros((nseg, 128, 2 * KH * 8), np.int16)  # wrapped A|B idx
    dstrel = np.full((nseg, 128, KT), -1.0, np.float64)
    for si, (e0, e1, n0, n1) in enumerate(segs):
        se, de = es[e0:e1], ed[e0:e1]
        srow = row_of_node[se]
        isA = srow < split
        for half, sel in ((0, isA), (1, ~isA)):
            sr = srow[sel] - (0 if half == 0 else split)
            dr = de[sel] - n0
            o2 = np.argsort(sr, kind="stable")
            sr, dr = sr[o2], dr[o2]
            L = len(sr)
            assert L <= KH * 128, f"half overflow {L}"
            tmp = np.zeros((KH, 128), np.int64)
            tmp.flat[:L] = sr
            # wrapped int16 layout: idx t of the half -> [t % 16, t // 16]
            w = tmp.reshape(KH * 128 // 16, 16).T  # [16, KH*8]
            idxw[si, :, half * KH * 8:(half + 1) * KH * 8] = np.tile(w, (8, 1))
            tmpf = np.full((KH, 128), -1.0, np.float64)
            tmpf.flat[:L] = dr
            dstrel[si, :, half * KH:(half + 1) * KH] = tmpf.T
    # partition-major layouts for single-DMA loads
    idxw_t = np.ascontiguousarray(idxw.transpose(1, 0, 2))        # [128,S,112]
    dstrel_t = np.ascontiguousarray(
        dstrel.astype(BF).transpose(1, 0, 2))                     # [128,S,KT]
    return idxw_t, dstrel_t, perm


def _build(NPAD, NSEG, split):
    nc = bacc.Bacc(None, target_bir_lowering=False, debug=False,
                   num_swdge_queues=4)

    xT = nc.declare_dram_parameter("xT", [IN_F, NPAD], BF16, isOutput=False)
    Wc = nc.declare_dram_parameter("Wc", [IN_F, 144], BF16, isOutput=False)
    biasqk = nc.declare_dram_parameter("biasqk", [128, 16], F32, isOutput=False)
    meanbv = nc.declare_dram_parameter("meanbv", [128, 16], F32, isOutput=False)
    iota = nc.declare_dram_parameter("iota", [128, 128], BF16, isOutput=False)
    idxw = nc.declare_dram_parameter("idxw", [128, NSEG, 2 * KH * 8], I16,
                                     isOutput=False)
    dstrel = nc.declare_dram_parameter("dstrel", [128, NSEG, KT], BF16,
                                       isOutput=False)
    out_ext = nc.declare_dram_parameter("out", [NSEG * 128, F], F32,
                                        isOutput=True)

    tab = nc.dram_tensor("tab", [NPAD, ROW], BF16)
    NB = NPAD // XB
    nloc = NSEG * 128
    kwblk = (nloc + WB - 1) // WB  # tab writes that contain local k windows

    with tile.TileContext(nc) as tc:
        with (
            tc.tile_pool(name="consts", bufs=1) as cpool,
            tc.tile_pool(name="xin", bufs=4) as xpool,
            tc.tile_pool(name="tabw", bufs=3) as tpool,

            tc.tile_pool(name="pa_psum", bufs=2, space="PSUM") as pa_psum,
            tc.tile_pool(name="gat", bufs=1) as gpool,
            tc.tile_pool(name="work", bufs=3) as wpool,
            tc.tile_pool(name="flush", bufs=3) as fpool,
            tc.tile_pool(name="tr_psum", bufs=2, space="PSUM") as tr_psum,
            tc.tile_pool(name="ke_psum", bufs=2, space="PSUM") as ke_psum,
            tc.tile_pool(name="seg_psum", bufs=2, space="PSUM") as spsum,
        ):
            nc.gpsimd.load_library(library_config.mlp)
            wc_t = cpool.tile([128, 144], BF16)
            nc.sync.dma_start(out=wc_t[:], in_=Wc[:, :])
            bqk_t = cpool.tile([128, 16], F32)
            nc.sync.dma_start(out=bqk_t[:], in_=biasqk[:, :])
            mbv_t = cpool.tile([128, 16], F32)
            nc.sync.dma_start(out=mbv_t[:], in_=meanbv[:, :])
            iota_t = cpool.tile([128, 128], BF16)
            nc.sync.dma_start(out=iota_t[:], in_=iota[:, :])
            ident = cpool.tile([128, 128], BF16)
            from concourse.masks import make_identity
            make_identity(nc, ident[:])

            # batched per-segment metadata: single DMAs, queued first
            itb = cpool.tile([128, NSEG * 2 * KH * 8], I16)
            it_dma = nc.sync.dma_start(
                out=itb[:].rearrange("p (s u) -> p s u", s=NSEG),
                in_=idxw[:, :, :])
            drb = cpool.tile([128, NSEG * KT], BF16)
            nc.sync.dma_start(
                out=drb[:].rearrange("p (s u) -> p s u", s=NSEG),
                in_=dstrel[:, :, :])
            # gather slots: every slot is written by every gather (dummy
            # idx 0 for pads), so no prefill is needed
            gt = [gpool.tile([128, KT * ROW], BF16, tag=f"g{i}",
                             name=f"g{i}")
                  for i in range(4)]
            # SBUF-resident k windows + output accumulator
            kwb = cpool.tile([128, kwblk * (WB // 128) * 8], BF16)
            outacc = cpool.tile([128, NSEG * F], F32)

            # ---------------- phase A: node table ----------------
            A_writes = []
            B_writes = []
            kw_copies = []
            for b in range(NB):
                xt = xpool.tile([128, XB], BF16)
                nc.sync.dma_start(out=xt[:], in_=xT[:, b * XB:(b + 1) * XB])
                for w in range(XB // WB):
                    nblk = WB // 128  # 8 blocks of 128 nodes per write
                    tt = tpool.tile([128, nblk * 144], BF16)
                    ttv = tt[:].rearrange("p (b u) -> p b u", b=nblk)
                    for jp in range(nblk // 2):
                        ps = pa_psum.tile([128, 288], F32)
                        for jj in range(2):
                            j = w * nblk + jp * 2 + jj
                            nc.tensor.matmul(
                                out=ps[:, jj * 144:(jj + 1) * 144],
                                lhsT=xt[:, j * 128:(j + 1) * 128],
                                rhs=wc_t[:], start=True, stop=True,
                            )
                        psv = ps[:].rearrange("p (b u) -> p b u", b=2)
                        tv2 = ttv[:, jp * 2:jp * 2 + 2, :]
                        if jp % 2 == 0:
                            nc.scalar.activation(
                                out=tv2[:, :, 0:128], in_=psv[:, :, 0:128],
                                func=mybir.ActivationFunctionType.Copy,
                            )
                        else:
                            nc.vector.tensor_copy(
                                out=tv2[:, :, 0:128], in_=psv[:, :, 0:128])
                        nc.vector.tensor_tensor(
                            out=tv2[:, :, 128:144], in0=psv[:, :, 128:144],
                            in1=bqk_t[:].unsqueeze(1).to_broadcast([128, 2, 16]),
                            op=mybir.AluOpType.add,
                        )
                    r0 = (b * XB // 128 + w * nblk) * 128
                    wi = nc.sync.dma_start(
                        out=tab[r0:r0 + WB, 0:144]
                            .rearrange("(b p) u -> p b u", b=nblk),
                        in_=ttv)
                    if r0 < split:
                        A_writes.append(wi)
                    if r0 + WB > split:
                        B_writes.append(wi)
                    # stash local-window k columns in SBUF for the edge phase
                    wb_i = r0 // WB
                    if wb_i < kwblk:
                        kc = nc.vector.tensor_copy(
                            out=kwb[:, wb_i * nblk * 8:(wb_i + 1) * nblk * 8]
                                .rearrange("p (b u) -> p b u", b=nblk),
                            in_=ttv[:, :, 136:144])
                        kw_copies.append(kc)

            # ---------------- edge phase ----------------
            # manual dependency edges: InstDMAGatherAnt APs are not tracked
            # by Tile, so order gathers vs. slot reuse + readers explicitly.
            # Pool executes in order, so only the first A/B gather needs the
            # table-ready fan-in.
            with nc.gpsimd.register("nidx") as rN:
                nc.gpsimd.reg_mov(rN, KH * 128)
                last_g_readers = {r: [] for r in range(4)}
                gA_list = [None] * NSEG
                gB_list = [None] * NSEG
                # DMASW sem lanes are assigned round-robin (8 lanes) over
                # Pool DMA instructions; queue_num must follow lane pairs
                # so each sem lane stays locked to one SWDGE queue.
                gcount = [0]
                prev_g = [None]

                def next_queue():
                    q = (gcount[0] // 2) % 4
                    gcount[0] += 1
                    return q

                def chain(g):
                    # pin scheduled order = emission order (Pool executes
                    # serially anyway) so the round-robin DMASW lane
                    # assignment stays aligned with queue_num
                    if prev_g[0] is not None:
                        add_dep_helper(g.ins, prev_g[0].ins, sync=True,
                                       reason="train order")
                    prev_g[0] = g

                def emit_gA(s):
                    g = gt[s % 4]
                    gA = nc.gpsimd.dma_gather(
                        out_ap=g[:, 0:KH * ROW]
                            .rearrange("p (b e) -> p b e", e=ROW),
                        in_ap=tab[0:split, :],
                        idxs_ap=itb[:, s * 112:s * 112 + KH * 8],
                        num_idxs=KH * 128, num_idxs_reg=rN, elem_size=ROW,
                        single_packet=False, queue_num=next_queue(),
                    )
                    chain(gA)
                    add_dep_helper(gA.ins, it_dma.ins, sync=True,
                                   reason="idx loaded")
                    for wr in A_writes:
                        add_dep_helper(gA.ins, wr.ins, sync=True,
                                       reason="tabA written")
                    for rd in last_g_readers[s % 4]:
                        add_dep_helper(gA.ins, rd.ins, sync=True,
                                       reason="slot WAR")
                    gA_list[s] = gA

                def emit_gB(s):
                    g = gt[s % 4]
                    gB = nc.gpsimd.dma_gather(
                        out_ap=g[:, KH * ROW:]
                            .rearrange("p (b e) -> p b e", e=ROW),
                        in_ap=tab[split:NPAD, :],
                        idxs_ap=itb[:, s * 112 + KH * 8:(s + 1) * 112],
                        num_idxs=KH * 128, num_idxs_reg=rN, elem_size=ROW,
                        single_packet=False, queue_num=next_queue(),
                    )
                    chain(gB)
                    add_dep_helper(gB.ins, it_dma.ins, sync=True,
                                   reason="idx loaded")
                    for wr in B_writes:
                        add_dep_helper(gB.ins, wr.ins, sync=True,
                                       reason="tabB written")
                    for rd in last_g_readers[s % 4]:
                        add_dep_helper(gB.ins, rd.ins, sync=True,
                                       reason="slot WAR")
                    gB_list[s] = gB

                for s in range(min(4, NSEG)):
                    emit_gA(s)

                for s in range(NSEG):
                    emit_gB(s)
                    g = gt[s % 4]
                    gA, gB = gA_list[s], gB_list[s]
                    kw = kwb[:, s * 8:(s + 1) * 8]
                    dr = drb[:, s * KT:(s + 1) * KT]

                    # one-hot S_T [e, n] per chunk slot
                    st = wpool.tile([128, KT * 128], BF16, tag="st")
                    nc.vector.tensor_tensor(
                        out=st[:].rearrange("p (c n) -> p c n", c=KT),
                        in0=dr.unsqueeze(2).to_broadcast([128, KT, 128]),
                        in1=iota_t[:].unsqueeze(1).to_broadcast([128, KT, 128]),
                        op=mybir.AluOpType.is_equal,
                    )
                    # S_node = transpose(S_T) per chunk, via PE + copy
                    sn = wpool.tile([128, KT * 128], BF16, tag="sn")
                    for q4 in range(KT // 2):
                        trp = tr_psum.tile([128, 256], BF16, tag="trp")
                        for jj in range(2):
                            j = q4 * 2 + jj
                            nc.tensor.transpose(
                                out=trp[:, jj * 128:(jj + 1) * 128],
                                in_=st[:, j * 128:(j + 1) * 128],
                                identity=ident[:],
                            )
                        nc.any.tensor_copy(
                            out=sn[:, q4 * 256:(q4 + 1) * 256], in_=trp[:])
                    # k[dst] per edge via one-hot matmul
                    keps = ke_psum.tile([128, KT * 8], F32, tag="keps")
                    for j in range(KT):
                        nc.tensor.matmul(
                            out=keps[:, j * 8:(j + 1) * 8],
                            lhsT=sn[:, j * 128:(j + 1) * 128], rhs=kw,
                            start=True, stop=True,
                        )

                    # coeff = q[src] + k[dst]
                    co = wpool.tile([128, KT * 8], F32, tag="co")
                    gv = g[:].rearrange("p (c u) -> p c u", c=KT)
                    co_op = nc.vector.tensor_tensor(
                        out=co[:].rearrange("p (c h) -> p c h", c=KT),
                        in0=gv[:, :, 128:136],
                        in1=keps[:].rearrange("p (c h) -> p c h", c=KT),
                        op=mybir.AluOpType.add,
                    )
                    add_dep_helper(co_op.ins, gA.ins, sync=True, reason="gathered")
                    add_dep_helper(co_op.ins, gB.ins, sync=True, reason="gathered")
                    # ex = exp(lrelu(coeff)) = max(exp(x), exp(0.2x))
                    ex1 = wpool.tile([128, KT * 8], BF16, tag="ex1")
                    nc.scalar.activation(out=ex1[:], in_=co[:],
                                         func=mybir.ActivationFunctionType.Exp)
                    ex2 = wpool.tile([128, KT * 8], BF16, tag="ex2")
                    nc.scalar.activation(out=ex2[:], in_=co[:],
                                         func=mybir.ActivationFunctionType.Exp,
                                         scale=0.2)

                    mt2 = wpool.tile([128, KT * 136], BF16, tag="mt2")
                    mv = mt2[:].rearrange("p (c u) -> p c u", c=KT)
                    nc.vector.tensor_tensor(
                        out=mv[:, :, 128:136],
                        in0=ex1[:].rearrange("p (c h) -> p c h", c=KT),
                        in1=ex2[:].rearrange("p (c h) -> p c h", c=KT),
                        op=mybir.AluOpType.max,
                    )
                    mm_op = nc.vector.tensor_tensor(
                        out=mv[:, :, 0:128].rearrange("p c (h f) -> p c h f", h=H),
                        in0=gv[:, :, 0:128].rearrange("p c (h f) -> p c h f", h=H),
                        in1=mv[:, :, 128:136].unsqueeze(3)
                            .to_broadcast([128, KT, H, F]),
                        op=mybir.AluOpType.mult,
                    )
                    add_dep_helper(mm_op.ins, gA.ins, sync=True, reason="gathered")
                    add_dep_helper(mm_op.ins, gB.ins, sync=True, reason="gathered")
                    last_g_readers[s % 4] = [co_op, mm_op]

                    ps = spsum.tile([128, 136], F32, tag="segps")
                    for j in range(KT):
                        nc.tensor.matmul(
                            out=ps[:], lhsT=st[:, j * 128:(j + 1) * 128],
                            rhs=mt2[:, j * 136:(j + 1) * 136],
                            start=(j == 0), stop=(j == KT - 1),
                        )

                    # flush into the SBUF output accumulator
                    den = fpool.tile([128, 8], F32, tag="den")
                    nc.scalar.activation(out=den[:], in_=ps[:, 128:136],
                                         func=mybir.ActivationFunctionType.Copy,
                                         scale=8.0, bias=1e-30)
                    rden = fpool.tile([128, 8], F32, tag="rden")
                    nc.vector.reciprocal(out=rden[:], in_=den[:])
                    vt = fpool.tile([128, 128], F32, tag="vt")
                    nc.vector.tensor_tensor(
                        out=vt[:].rearrange("p (f h) -> p f h", h=H)
                            .rearrange("p f h -> p h f"),
                        in0=ps[:, 0:128].rearrange("p (h f) -> p h f", f=F),
                        in1=rden[:].unsqueeze(2).to_broadcast([128, H, F]),
                        op=mybir.AluOpType.mult,
                    )
                    vo = fpool.tile([128, F], F32, tag="vo")
                    nc.vector.reduce_sum(
                        out=vo[:], in_=vt[:].rearrange("p (f h) -> p f h", h=H),
                        axis=mybir.AxisListType.X,
                    )
                    nc.vector.tensor_tensor(
                        out=outacc[:, s * F:(s + 1) * F],
                        in0=vo[:], in1=mbv_t[:], op=mybir.AluOpType.add)

                    if s + 4 < NSEG:
                        emit_gA(s + 4)

            nc.sync.dma_start(
                out=out_ext[:, :].rearrange("(s p) f -> p s f", p=128),
                in_=outacc[:].rearrange("p (s f) -> p s f", s=NSEG))
    nc.finalize()
    return nc


def _prep_inputs(x, src, dst, Wv, bv, Wq, bq, Wk, bk):
    Wq_eff = (Wv @ Wq).astype(np.float32)
    bq_eff = (bv @ Wq + bq).astype(np.float32)
    Wk_eff = (Wv @ Wk).astype(np.float32)
    bk_eff = (bv @ Wk + bk).astype(np.float32)
    Wc = np.concatenate([Wv, Wq_eff, Wk_eff], axis=1).astype(BF)
    biasqk = np.broadcast_to(
        np.concatenate([bq_eff, bk_eff]).astype(np.float32), (128, 16)).copy()
    meanbv = np.broadcast_to(
        bv.reshape(H, F).mean(axis=0).astype(np.float32), (128, F)).copy()
    iota = np.broadcast_to(
        np.arange(128, dtype=np.float32), (128, 128)).astype(BF).copy()

    edges = [_edges_of_core(src, dst, c) for c in range(C)]

    # iterate: the A/B split position depends on NSEG (local slots come
    # first in the table), which depends on the per-half capacities.
    NSEG = (NL * (E // N) + CAP - 1) // CAP + 2  # initial guess
    for _ in range(10):
        nloc = NSEG * 128
        NPAD = ((nloc + (N - NL) + XB - 1) // XB) * XB
        split = (min(32640, NPAD // 2) // 128) * 128
        all_segs = []
        for c in range(C):
            es, ed, counts = edges[c]
            lo = c * NL
            # row of src: local srcs are always < nloc <= split -> A;
            # foreign srcs: position in foreign order decides the half.
            pos = np.where(es < lo, es, es - NL)  # foreign position
            frow = nloc + pos
            is_local = (es >= lo) & (es < lo + NL)
            isA = is_local | (frow < split)
            all_segs.append(_segment(es, ed, counts, isA))
        new_NSEG = max(len(s) for s in all_segs)
        if new_NSEG == NSEG:
            break
        NSEG = new_NSEG  # grow or shrink toward the fixpoint
    else:
        # no fixpoint: grow-only until the layout fits (extra dummy
        # segments are harmless)
        for _ in range(10):
            nloc = NSEG * 128
            NPAD = ((nloc + (N - NL) + XB - 1) // XB) * XB
            split = (min(32640, NPAD // 2) // 128) * 128
            all_segs = []
            for c in range(C):
                es, ed, counts = edges[c]
                lo = c * NL
                pos = np.where(es < lo, es, es - NL)
                frow = nloc + pos
                is_local = (es >= lo) & (es < lo + NL)
                isA = is_local | (frow < split)
                all_segs.append(_segment(es, ed, counts, isA))
            new_NSEG = max(len(s) for s in all_segs)
            if new_NSEG <= NSEG:
                break
            NSEG = new_NSEG
    assert NSEG * 128 <= split, (
        f"local segment slots ({NSEG * 128}) exceed the A half ({split})")

    xf = x.astype(np.float32)
    in_maps = []
    perms = []
    degs = []
    for c in range(C):
        es, ed, counts = edges[c]
        segs = all_segs[c]
        idxw_, dstrel_, perm = _core_arrays(es, ed, segs, NSEG, c,
                                            split, NPAD)
        xTc = xf[perm].T.astype(BF).copy()
        in_maps.append({
            "xT": xTc, "Wc": Wc, "biasqk": biasqk, "meanbv": meanbv,
            "iota": iota, "idxw": idxw_, "dstrel": dstrel_,
        })
        perms.append(perm)
        degs.append(counts)
    return in_maps, perms, degs, NSEG, NPAD, split


def kernel(x, src, dst, Wv, bv, Wq, bq, Wk, bk):
    x = np.asarray(x, np.float32)
    src = np.asarray(src, np.int32)
    dst = np.asarray(dst, np.int32)
    Wv, bv = np.asarray(Wv, np.float32), np.asarray(bv, np.float32)
    Wq, bq = np.asarray(Wq, np.float32), np.asarray(bq, np.float32)
    Wk, bk = np.asarray(Wk, np.float32), np.asarray(bk, np.float32)

    in_maps, perms, degs, NSEG, NPAD, split = _prep_inputs(
        x, src, dst, Wv, bv, Wq, bq, Wk, bk)
    nc = _build(NPAD, NSEG, split)
    res = run_bass_kernel_spmd(nc, in_maps, core_ids=list(range(C)))
    return assemble(res.results, perms, degs)


def assemble(results, perms, degs):
    out = np.zeros((N, F), np.float32)
    for c in range(C):
        dev = np.asarray(results[c]["out"])  # [NSEG*128, F]
        nrows = dev.shape[0]
        lo = c * NL
        rows = perms[c][:nrows]
        local = (rows >= lo) & (rows < lo + NL)
        # segment-slot rows that map to real local nodes with degree > 0
        rl = rows[local]
        dl = dev[:nrows][local]
        keep = degs[c][rl - lo] > 0
        out[rl[keep]] = dl[keep]
    return out


# revision 18
# speedup vs baseline: 1.4929x; 1.4929x over previous
"""GAT layer on 8 Trainium2 NeuronCores (Bass/Tile, SPMD).

Sharding: nodes partitioned across the 8 cores; each edge is assigned to
the core owning its dst node, so edge-softmax and the scatter-sum are
core-local.  Weights are replicated and pre-composed on the host
(q = x @ (Wv Wq) + (bv Wq + bq), etc).  Each core computes the full node
table [h | q | k] (replicated compute instead of a halo all-gather), in a
per-core permutation that places its own dst nodes in segment-window
order.  Edges are processed in segments of <=128 dst nodes; h/q rows are
fetched with dma_gather (negative-index-masked two-pass for the int16
range) spread over the 4 SWDGE queues (Q7 core pairs), k[dst] is expanded
on the TensorEngine via the transposed one-hot, and the softmax-weighted
aggregation is a one-hot matmul into PSUM.

Perf structure: all per-segment metadata (gather indices, dst-relative
one-hot keys, k windows, outputs) lives in batched SBUF tiles loaded or
flushed in single DMAs, keeping the Sync HWDGE queue short.  The gather
train on the Pool engine is the critical path; A-half gathers start as
soon as the A-half of the table is written.
"""
import sys

for _p in ("/opt/trn_rl_repo",):
    if _p not in sys.path:
        sys.path.insert(0, _p)

import numpy as np
import ml_dtypes

import concourse.bass as bass
from concourse import bacc, tile, library_config
from concourse.tile import add_dep_helper
import concourse.mybir as mybir
from concourse.bass_utils import run_bass_kernel_spmd

F32 = mybir.dt.float32
BF16 = mybir.dt.bfloat16
I32 = mybir.dt.int32
I16 = mybir.dt.int16
BF = ml_dtypes.bfloat16

# problem constants
N = 50000
E = 800000
IN_F = 128
H = 8
F = 16
HF = H * F  # 128
C = 8            # cores
CAP = 12 * 128   # max edges per segment
KH = 7           # chunk slots per table half
KT = 2 * KH      # chunk slots per segment
ROW = 256        # table row units (bf16): [h 128 | q 8 | k 8 | pad]
XB = 2048        # phase-A x tile width
WB = 1024        # tab rows per write DMA
NL = N // C      # nodes per core


def _edges_of_core(src, dst, c):
    lo = c * NL
    m = (dst >= lo) & (dst < lo + NL)
    es = src[m].astype(np.int64)
    ed = (dst[m] - lo).astype(np.int64)
    order = np.lexsort((es, ed))
    es, ed = es[order], ed[order]
    counts = np.bincount(ed, minlength=NL)
    return es, ed, counts


def _segment(es, ed, counts, isA):
    """Greedy segmentation: window <=128 nodes, per-half <= KH*128 edges."""
    cum = np.concatenate([[0], np.cumsum(counts)])
    cumA = np.concatenate([[0], np.cumsum(isA.astype(np.int64))])[cum]
    segs = []
    n0 = 0
    HC = KH * 128
    while n0 < NL:
        e0 = cum[n0]
        a0 = cumA[n0]
        n_hi = min(n0 + 128, NL)
        # largest n1 with both halves within capacity
        n1 = n0
        for n in range(n0 + 1, n_hi + 1):
            na = cumA[n] - a0
            nb = (cum[n] - e0) - na
            if na > HC or nb > HC:
                break
            n1 = n
        if n1 == n0:
            raise ValueError("node exceeds half capacity")
        segs.append((int(e0), int(cum[n1]), n0, n1))
        n0 = n1
    return segs


def _core_arrays(es, ed, segs, nseg, c, split, npad):
    """Build per-core device arrays + the node permutation.

    Table rows (per core): [0, nseg*128) = segment-window slots of local
    nodes; [nseg*128, ...) = all other nodes; padded to npad.
    perm[row] = global node id occupying that row (-1 for pad slots).
    """
    lo = c * NL
    nloc = nseg * 128
    perm = np.full(npad, -1, np.int64)
    row_of_node = np.full(N, -1, np.int64)
    for si, (e0, e1, n0, n1) in enumerate(segs):
        nn = n1 - n0
        rows = si * 128 + np.arange(nn)
        perm[rows] = lo + n0 + np.arange(nn)
        row_of_node[lo + n0 + np.arange(nn)] = rows
    foreign = np.concatenate([np.arange(0, lo), np.arange(lo + NL, N)])
    perm[nloc:nloc + len(foreign)] = foreign
    row_of_node[foreign] = nloc + np.arange(len(foreign))

    # pad slots hold dummy VALID index 0 (row 0 fetched, killed by
    # dstrel=-1 downstream): every gather processes exactly KH*128 idxs,
    # keeping the NX decode's ring accounting and the Q7 ucode in lockstep
    # with the constant num_idxs register.
    idxw = np.zeros((nseg, 128, 2 * KH * 8), np.int16)  # wrapped A|B idx
    dstrel = np.full((nseg, 128, KT), -1.0, np.float64)
    for si, (e0, e1, n0, n1) in enumerate(segs):
        se, de = es[e0:e1], ed[e0:e1]
        srow = row_of_node[se]
        isA = srow < split
        for half, sel in ((0, isA), (1, ~isA)):
            sr = srow[sel] - (0 if half == 0 else split)
            dr = de[sel] - n0
            o2 = np.argsort(sr, kind="stable")
            sr, dr = sr[o2], dr[o2]
            L = len(sr)
            assert L <= KH * 128, f"half overflow {L}"
            # pad rows spread over distinct low rows (HBM bank parallel)
            tmp = np.arange(KH * 128, dtype=np.int64).reshape(KH, 128)
            tmp.flat[:L] = sr
            # wrapped int16 layout: idx t of the half -> [t % 16, t // 16]
            w = tmp.reshape(KH * 128 // 16, 16).T  # [16, KH*8]
            idxw[si, :, half * KH * 8:(half + 1) * KH * 8] = np.tile(w, (8, 1))
            tmpf = np.full((KH, 128), -1.0, np.float64)
            tmpf.flat[:L] = dr
            dstrel[si, :, half * KH:(half + 1) * KH] = tmpf.T
    # partition-major layouts for single-DMA loads
    idxw_t = np.ascontiguousarray(idxw.transpose(1, 0, 2))        # [128,S,112]
    dstrel_t = np.ascontiguousarray(
        dstrel.astype(BF).transpose(1, 0, 2))                     # [128,S,KT]
    return idxw_t, dstrel_t, perm


def _build(NPAD, NSEG, split):
    nc = bacc.Bacc(None, target_bir_lowering=False, debug=False,
                   num_swdge_queues=4)

    xT = nc.declare_dram_parameter("xT", [IN_F, NPAD], BF16, isOutput=False)
    Wc = nc.declare_dram_parameter("Wc", [IN_F, 144], BF16, isOutput=False)
    biasqk = nc.declare_dram_parameter("biasqk", [128, 16], F32, isOutput=False)
    meanbv = nc.declare_dram_parameter("meanbv", [128, 16], F32, isOutput=False)
    iota = nc.declare_dram_parameter("iota", [128, 128], BF16, isOutput=False)
    idxw = nc.declare_dram_parameter("idxw", [128, NSEG, 2 * KH * 8], I16,
                                     isOutput=False)
    dstrel = nc.declare_dram_parameter("dstrel", [128, NSEG, KT], BF16,
                                       isOutput=False)
    out_ext = nc.declare_dram_parameter("out", [NSEG * 128, F], F32,
                                        isOutput=True)

    tab = nc.dram_tensor("tab", [NPAD, ROW], BF16)
    NB = NPAD // XB
    nloc = NSEG * 128
    kwblk = (nloc + WB - 1) // WB  # tab writes that contain local k windows

    with tile.TileContext(nc) as tc:
        with (
            tc.tile_pool(name="consts", bufs=1) as cpool,
            tc.tile_pool(name="xin", bufs=4) as xpool,
            tc.tile_pool(name="tabw", bufs=3) as tpool,

            tc.tile_pool(name="pa_psum", bufs=2, space="PSUM") as pa_psum,
            tc.tile_pool(name="gat", bufs=1) as gpool,
            tc.tile_pool(name="work", bufs=3) as wpool,
            tc.tile_pool(name="flush", bufs=3) as fpool,
            tc.tile_pool(name="tr_psum", bufs=2, space="PSUM") as tr_psum,
            tc.tile_pool(name="ke_psum", bufs=2, space="PSUM") as ke_psum,
            tc.tile_pool(name="seg_psum", bufs=2, space="PSUM") as spsum,
        ):
            nc.gpsimd.load_library(library_config.mlp)
            wc_t = cpool.tile([128, 144], BF16)
            nc.sync.dma_start(out=wc_t[:], in_=Wc[:, :])
            bqk_t = cpool.tile([128, 16], F32)
            nc.sync.dma_start(out=bqk_t[:], in_=biasqk[:, :])
            mbv_t = cpool.tile([128, 16], F32)
            nc.sync.dma_start(out=mbv_t[:], in_=meanbv[:, :])
            iota_t = cpool.tile([128, 128], BF16)
            nc.sync.dma_start(out=iota_t[:], in_=iota[:, :])
            ident = cpool.tile([128, 128], BF16)
            from concourse.masks import make_identity
            make_identity(nc, ident[:])

            # batched per-segment metadata: single DMAs, queued first
            itb = cpool.tile([128, NSEG * 2 * KH * 8], I16)
            it_dma = nc.sync.dma_start(
                out=itb[:].rearrange("p (s u) -> p s u", s=NSEG),
                in_=idxw[:, :, :])
            drb = cpool.tile([128, NSEG * KT], BF16)
            nc.sync.dma_start(
                out=drb[:].rearrange("p (s u) -> p s u", s=NSEG),
                in_=dstrel[:, :, :])
            # gather slots: every slot is written by every gather (dummy
            # idx 0 for pads), so no prefill is needed
            gt = [gpool.tile([128, KT * ROW], BF16, tag=f"g{i}",
                             name=f"g{i}")
                  for i in range(4)]
            # SBUF-resident k windows + output accumulator
            kwb = cpool.tile([128, kwblk * (WB // 128) * 8], BF16)
            outacc = cpool.tile([128, NSEG * F], F32)

            # ---------------- phase A: node table ----------------
            A_writes = []
            B_writes = []
            kw_copies = []
            for b in range(NB):
                xt = xpool.tile([128, XB], BF16)
                nc.sync.dma_start(out=xt[:], in_=xT[:, b * XB:(b + 1) * XB])
                for w in range(XB // WB):
                    nblk = WB // 128  # 8 blocks of 128 nodes per write
                    tt = tpool.tile([128, nblk * 144], BF16)
                    ttv = tt[:].rearrange("p (b u) -> p b u", b=nblk)
                    for jp in range(nblk // 2):
                        ps = pa_psum.tile([128, 288], F32)
                        for jj in range(2):
                            j = w * nblk + jp * 2 + jj
                            nc.tensor.matmul(
                                out=ps[:, jj * 144:(jj + 1) * 144],
                                lhsT=xt[:, j * 128:(j + 1) * 128],
                                rhs=wc_t[:], start=True, stop=True,
                            )
                        psv = ps[:].rearrange("p (b u) -> p b u", b=2)
                        tv2 = ttv[:, jp * 2:jp * 2 + 2, :]
                        if jp % 2 == 0:
                            nc.scalar.activation(
                                out=tv2[:, :, 0:128], in_=psv[:, :, 0:128],
                                func=mybir.ActivationFunctionType.Copy,
                            )
                        else:
                            nc.vector.tensor_copy(
                                out=tv2[:, :, 0:128], in_=psv[:, :, 0:128])
                        nc.vector.tensor_tensor(
                            out=tv2[:, :, 128:144], in0=psv[:, :, 128:144],
                            in1=bqk_t[:].unsqueeze(1).to_broadcast([128, 2, 16]),
                            op=mybir.AluOpType.add,
                        )
                    r0 = (b * XB // 128 + w * nblk) * 128
                    wi = nc.sync.dma_start(
                        out=tab[r0:r0 + WB, 0:144]
                            .rearrange("(b p) u -> p b u", b=nblk),
                        in_=ttv)
                    if r0 < split:
                        A_writes.append(wi)
                    if r0 + WB > split:
                        B_writes.append(wi)
                    # stash local-window k columns in SBUF for the edge phase
                    wb_i = r0 // WB
                    if wb_i < kwblk:
                        kc = nc.vector.tensor_copy(
                            out=kwb[:, wb_i * nblk * 8:(wb_i + 1) * nblk * 8]
                                .rearrange("p (b u) -> p b u", b=nblk),
                            in_=ttv[:, :, 136:144])
                        kw_copies.append(kc)

            # ---------------- edge phase ----------------
            # manual dependency edges: InstDMAGatherAnt APs are not tracked
            # by Tile, so order gathers vs. slot reuse + readers explicitly.
            # Pool executes in order, so only the first A/B gather needs the
            # table-ready fan-in.
            with nc.gpsimd.register("nidx") as rN:
                nc.gpsimd.reg_mov(rN, KH * 128)
                last_g_readers = {r: [] for r in range(4)}
                gA_list = [None] * NSEG
                gB_list = [None] * NSEG
                # DMASW sem lanes are assigned round-robin (8 lanes) over
                # Pool DMA instructions; queue_num must follow lane pairs
                # so each sem lane stays locked to one SWDGE queue.
                gcount = [0]
                prev_g = [None]

                def next_queue():
                    q = (gcount[0] // 2) % 4
                    gcount[0] += 1
                    return q

                def chain(g):
                    # pin scheduled order = emission order (Pool executes
                    # serially anyway) so the round-robin DMASW lane
                    # assignment stays aligned with queue_num
                    if prev_g[0] is not None:
                        add_dep_helper(g.ins, prev_g[0].ins, sync=False,
                                       reason="train order")
                    prev_g[0] = g

                def emit_gA(s):
                    g = gt[s % 4]
                    gA = nc.gpsimd.dma_gather(
                        out_ap=g[:, 0:KH * ROW]
                            .rearrange("p (b e) -> p b e", e=ROW),
                        in_ap=tab[0:split, :],
                        idxs_ap=itb[:, s * 112:s * 112 + KH * 8],
                        num_idxs=KH * 128, num_idxs_reg=rN, elem_size=ROW,
                        single_packet=False, queue_num=next_queue(),
                    )
                    chain(gA)
                    add_dep_helper(gA.ins, it_dma.ins, sync=True,
                                   reason="idx loaded")
                    for wr in A_writes:
                        add_dep_helper(gA.ins, wr.ins, sync=True,
                                       reason="tabA written")
                    for rd in last_g_readers[s % 4]:
                        add_dep_helper(gA.ins, rd.ins, sync=True,
                                       reason="slot WAR")
                    gA_list[s] = gA

                def emit_gB(s):
                    g = gt[s % 4]
                    gB = nc.gpsimd.dma_gather(
                        out_ap=g[:, KH * ROW:]
                            .rearrange("p (b e) -> p b e", e=ROW),
                        in_ap=tab[split:NPAD, :],
                        idxs_ap=itb[:, s * 112 + KH * 8:(s + 1) * 112],
                        num_idxs=KH * 128, num_idxs_reg=rN, elem_size=ROW,
                        single_packet=False, queue_num=next_queue(),
                    )
                    chain(gB)
                    add_dep_helper(gB.ins, it_dma.ins, sync=True,
                                   reason="idx loaded")
                    for wr in B_writes:
                        add_dep_helper(gB.ins, wr.ins, sync=True,
                                       reason="tabB written")
                    for rd in last_g_readers[s % 4]:
                        add_dep_helper(gB.ins, rd.ins, sync=True,
                                       reason="slot WAR")
                    gB_list[s] = gB

                for s in range(min(4, NSEG)):
                    emit_gA(s)

                for s in range(NSEG):
                    emit_gB(s)
                    g = gt[s % 4]
                    gA, gB = gA_list[s], gB_list[s]
                    kw = kwb[:, s * 8:(s + 1) * 8]
                    dr = drb[:, s * KT:(s + 1) * KT]

                    # one-hot S_T [e, n] per chunk slot
                    st = wpool.tile([128, KT * 128], BF16, tag="st")
                    nc.vector.tensor_tensor(
                        out=st[:].rearrange("p (c n) -> p c n", c=KT),
                        in0=dr.unsqueeze(2).to_broadcast([128, KT, 128]),
                        in1=iota_t[:].unsqueeze(1).to_broadcast([128, KT, 128]),
                        op=mybir.AluOpType.is_equal,
                    )
                    # S_node = transpose(S_T) per chunk, via PE + copy
                    sn = wpool.tile([128, KT * 128], BF16, tag="sn")
                    for q4 in range(KT // 2):
                        trp = tr_psum.tile([128, 256], BF16, tag="trp")
                        for jj in range(2):
                            j = q4 * 2 + jj
                            nc.tensor.transpose(
                                out=trp[:, jj * 128:(jj + 1) * 128],
                                in_=st[:, j * 128:(j + 1) * 128],
                                identity=ident[:],
                            )
                        nc.any.tensor_copy(
                            out=sn[:, q4 * 256:(q4 + 1) * 256], in_=trp[:])
                    # k[dst] per edge via one-hot matmul
                    keps = ke_psum.tile([128, KT * 8], F32, tag="keps")
                    for j in range(KT):
                        nc.tensor.matmul(
                            out=keps[:, j * 8:(j + 1) * 8],
                            lhsT=sn[:, j * 128:(j + 1) * 128], rhs=kw,
                            start=True, stop=True,
                        )

                    # coeff = q[src] + k[dst]
                    co = wpool.tile([128, KT * 8], F32, tag="co")
                    gv = g[:].rearrange("p (c u) -> p c u", c=KT)
                    co_op = nc.vector.tensor_tensor(
                        out=co[:].rearrange("p (c h) -> p c h", c=KT),
                        in0=gv[:, :, 128:136],
                        in1=keps[:].rearrange("p (c h) -> p c h", c=KT),
                        op=mybir.AluOpType.add,
                    )
                    add_dep_helper(co_op.ins, gA.ins, sync=True, reason="gathered")
                    add_dep_helper(co_op.ins, gB.ins, sync=True, reason="gathered")
                    # ex = exp(lrelu(coeff)) = max(exp(x), exp(0.2x))
                    ex1 = wpool.tile([128, KT * 8], BF16, tag="ex1")
                    nc.scalar.activation(out=ex1[:], in_=co[:],
                                         func=mybir.ActivationFunctionType.Exp)
                    ex2 = wpool.tile([128, KT * 8], BF16, tag="ex2")
                    nc.scalar.activation(out=ex2[:], in_=co[:],
                                         func=mybir.ActivationFunctionType.Exp,
                                         scale=0.2)

                    mt2 = wpool.tile([128, KT * 136], BF16, tag="mt2")
                    mv = mt2[:].rearrange("p (c u) -> p c u", c=KT)
                    nc.vector.tensor_tensor(
                        out=mv[:, :, 128:136],
                        in0=ex1[:].rearrange("p (c h) -> p c h", c=KT),
                        in1=ex2[:].rearrange("p (c h) -> p c h", c=KT),
                        op=mybir.AluOpType.max,
                    )
                    mm_op = nc.vector.tensor_tensor(
                        out=mv[:, :, 0:128].rearrange("p c (h f) -> p c h f", h=H),
                        in0=gv[:, :, 0:128].rearrange("p c (h f) -> p c h f", h=H),
                        in1=mv[:, :, 128:136].unsqueeze(3)
                            .to_broadcast([128, KT, H, F]),
                        op=mybir.AluOpType.mult,
                    )
                    add_dep_helper(mm_op.ins, gA.ins, sync=True, reason="gathered")
                    add_dep_helper(mm_op.ins, gB.ins, sync=True, reason="gathered")
                    last_g_readers[s % 4] = [co_op, mm_op]

                    ps = spsum.tile([128, 136], F32, tag="segps")
                    for j in range(KT):
                        nc.tensor.matmul(
                            out=ps[:], lhsT=st[:, j * 128:(j + 1) * 128],
                            rhs=mt2[:, j * 136:(j + 1) * 136],
                            start=(j == 0), stop=(j == KT - 1),
                        )

                    # flush into the SBUF output accumulator
                    den = fpool.tile([128, 8], F32, tag="den")
                    nc.scalar.activation(out=den[:], in_=ps[:, 128:136],
                                         func=mybir.ActivationFunctionType.Copy,
                                         scale=8.0, bias=1e-30)
                    rden = fpool.tile([128, 8], F32, tag="rden")
                    nc.vector.reciprocal(out=rden[:], in_=den[:])
                    vt = fpool.tile([128, 128], F32, tag="vt")
                    nc.vector.tensor_tensor(
                        out=vt[:].rearrange("p (f h) -> p f h", h=H)
                            .rearrange("p f h -> p h f"),
                        in0=ps[:, 0:128].rearrange("p (h f) -> p h f", f=F),
                        in1=rden[:].unsqueeze(2).to_broadcast([128, H, F]),
                        op=mybir.AluOpType.mult,
                    )
                    vo = fpool.tile([128, F], F32, tag="vo")
                    nc.vector.reduce_sum(
                        out=vo[:], in_=vt[:].rearrange("p (f h) -> p f h", h=H),
                        axis=mybir.AxisListType.X,
                    )
                    nc.vector.tensor_tensor(
                        out=outacc[:, s * F:(s + 1) * F],
                        in0=vo[:], in1=mbv_t[:], op=mybir.AluOpType.add)

                    if s + 4 < NSEG:
                        emit_gA(s + 4)

            nc.sync.dma_start(
                out=out_ext[:, :].rearrange("(s p) f -> p s f", p=128),
                in_=outacc[:].rearrange("p (s f) -> p s f", s=NSEG))
    nc.finalize()
    return nc


def _prep_inputs(x, src, dst, Wv, bv, Wq, bq, Wk, bk):
    Wq_eff = (Wv @ Wq).astype(np.float32)
    bq_eff = (bv @ Wq + bq).astype(np.float32)
    Wk_eff = (Wv @ Wk).astype(np.float32)
    bk_eff = (bv @ Wk + bk).astype(np.float32)
    Wc = np.concatenate([Wv, Wq_eff, Wk_eff], axis=1).astype(BF)
    biasqk = np.broadcast_to(
        np.concatenate([bq_eff, bk_eff]).astype(np.float32), (128, 16)).copy()
    meanbv = np.broadcast_to(
        bv.reshape(H, F).mean(axis=0).astype(np.float32), (128, F)).copy()
    iota = np.broadcast_to(
        np.arange(128, dtype=np.float32), (128, 128)).astype(BF).copy()

    edges = [_edges_of_core(src, dst, c) for c in range(C)]

    # iterate: the A/B split position depends on NSEG (local slots come
    # first in the table), which depends on the per-half capacities.
    NSEG = (NL * (E // N) + CAP - 1) // CAP + 2  # initial guess
    for _ in range(10):
        nloc = NSEG * 128
        NPAD = ((nloc + (N - NL) + XB - 1) // XB) * XB
        split = (min(32640, NPAD // 2) // 128) * 128
        all_segs = []
        for c in range(C):
            es, ed, counts = edges[c]
            lo = c * NL
            # row of src: local srcs are always < nloc <= split -> A;
            # foreign srcs: position in foreign order decides the half.
            pos = np.where(es < lo, es, es - NL)  # foreign position
            frow = nloc + pos
            is_local = (es >= lo) & (es < lo + NL)
            isA = is_local | (frow < split)
            all_segs.append(_segment(es, ed, counts, isA))
        new_NSEG = max(len(s) for s in all_segs)
        if new_NSEG == NSEG:
            break
        NSEG = new_NSEG  # grow or shrink toward the fixpoint
    else:
        # no fixpoint: grow-only until the layout fits (extra dummy
        # segments are harmless)
        for _ in range(10):
            nloc = NSEG * 128
            NPAD = ((nloc + (N - NL) + XB - 1) // XB) * XB
            split = (min(32640, NPAD // 2) // 128) * 128
            all_segs = []
            for c in range(C):
                es, ed, counts = edges[c]
                lo = c * NL
                pos = np.where(es < lo, es, es - NL)
                frow = nloc + pos
                is_local = (es >= lo) & (es < lo + NL)
                isA = is_local | (frow < split)
                all_segs.append(_segment(es, ed, counts, isA))
            new_NSEG = max(len(s) for s in all_segs)
            if new_NSEG <= NSEG:
                break
            NSEG = new_NSEG
    assert NSEG * 128 <= split, (
        f"local segment slots ({NSEG * 128}) exceed the A half ({split})")

    xf = x.astype(np.float32)
    in_maps = []
    perms = []
    degs = []
    for c in range(C):
        es, ed, counts = edges[c]
        segs = all_segs[c]
        idxw_, dstrel_, perm = _core_arrays(es, ed, segs, NSEG, c,
                                            split, NPAD)
        xTc = xf[perm].T.astype(BF).copy()
        in_maps.append({
            "xT": xTc, "Wc": Wc, "biasqk": biasqk, "meanbv": meanbv,
            "iota": iota, "idxw": idxw_, "dstrel": dstrel_,
        })
        perms.append(perm)
        degs.append(counts)
    return in_maps, perms, degs, NSEG, NPAD, split


def kernel(x, src, dst, Wv, bv, Wq, bq, Wk, bk):
    x = np.asarray(x, np.float32)
    src = np.asarray(src, np.int32)
    dst = np.asarray(dst, np.int32)
    Wv, bv = np.asarray(Wv, np.float32), np.asarray(bv, np.float32)
    Wq, bq = np.asarray(Wq, np.float32), np.asarray(bq, np.float32)
    Wk, bk = np.asarray(Wk, np.float32), np.asarray(bk, np.float32)

    in_maps, perms, degs, NSEG, NPAD, split = _prep_inputs(
        x, src, dst, Wv, bv, Wq, bq, Wk, bk)
    nc = _build(NPAD, NSEG, split)
    res = run_bass_kernel_spmd(nc, in_maps, core_ids=list(range(C)))
    return assemble(res.results, perms, degs)


def assemble(results, perms, degs):
    out = np.zeros((N, F), np.float32)
    for c in range(C):
        dev = np.asarray(results[c]["out"])  # [NSEG*128, F]
        nrows = dev.shape[0]
        lo = c * NL
        rows = perms[c][:nrows]
        local = (rows >= lo) & (rows < lo + NL)
        # segment-slot rows that map to real local nodes with degree > 0
        rl = rows[local]
        dl = dev[:nrows][local]
        keep = degs[c][rl - lo] > 0
        out[rl[keep]] = dl[keep]
    return out
